# revision 7
# baseline (speedup 1.0000x reference)
"""Gumbel top-k (sequential masking) Trainium2 kernel, v2.

Problem: B=64 rows, N=16384, K=16 sequential top-1+mask steps.
  noisy = logits + gumbel; per step j: soft_j = softmax(noisy_masked/TAU),
  select argmax, mask it; outputs st (one-hot) and softs, each [K, B, N].

v2 strategy (data-parallel over batch, 8 rows/core on 8 cores):
  - Outputs shrink: softs emitted bf16 (rel err ~2e-3 << 2e-2 gate), st
    emitted u8 {0,1}. DRAM layout is partition-major [P, K*1024] so group
    DMAs have fat per-partition contiguous runs (few descriptors).
  - st = zero-planes DMA'd from an SBUF zero region at kernel start +
    128 scattered one-bytes (one per row*step) via a [P,1] indirect DMA.
    No per-plane compute at all.
  - softs plane j = e_src * r_j (ACT activation / DVE tensor_scalar,
    bf16 out) where e_src is a group-masked e tile: e0 -> e4 -> e8 -> e12
    via match_replace (4 keys each), and planes 12..15 get exact tiles
    e13/e14/e15 (1 extra key each) so the LAST group needs no fix-up.
    Planes 4g+t (groups 0..2, t>0) are written unmasked-within-group and
    the <=3 stale positions per row are zeroed IN DRAM afterward by tiny
    [P,1] indirect scatters (multi-offset indirect DMA is broken on HW;
    [P,1] is proven).
  - Scatter offsets are data-dependent: per-partition top-8-per-half
    values AND indices (max8 + max_index), stream_shuffled so every
    partition holds its row's 16 candidate (value, gidx) pairs; row
    top-16 by value (max8/match_replace/max8) + slots via max_index;
    then two chained [P,1] gathers through small DRAM stagings resolve
    rank-ordered global indices per partition (diagonal extraction via
    host-precomputed constant offset maps).
  - DRAM write-after-write ordering (zero-fill/plane DMA before scatter)
    is enforced through Tile: a tiny memset into the DMA-read region
    (W-A-R => waits DMA completion sem) then a value-preserving rewrite
    of the consumer's offset tile.
"""

import numpy as np
from contextlib import ExitStack

import concourse.bacc as bacc
import concourse.bass as bass
import concourse.mybir as mybir
import concourse.tile as tile
from concourse.bass_utils import run_bass_kernel_spmd

F32 = mybir.dt.float32
BF16 = mybir.dt.bfloat16
U8 = mybir.dt.uint8
U32 = mybir.dt.uint32
AF = mybir.ActivationFunctionType
OP = mybir.AluOpType

B, N, NCORES = 64, 16384, 8
R = B // NCORES          # rows per core = 8
QP = 16                  # partitions per row
FREE = N // QP           # 1024
P = 128                  # SBUF partitions
H = FREE // 2            # 512
INV_TAU = 1.5            # 1/(2/3), exact in fp32
K16 = 16
KF = K16 * FREE          # 16384 elements per partition of each output

# group maps for the soft-zero scatters (G=4; groups 0..2 scatter-fixed)
MREL = [0, 0, 1, 0, 1, 2]   # item -> rank offset within group
TMAP = [1, 2, 2, 3, 3, 3]   # item -> plane offset within group

_module_cache = {}


def _host_consts():
    p = np.arange(P)
    i16 = p % 16
    cq1024 = (i16 * 1024).astype(np.uint32)
    crow16 = (p // 16 * 16).astype(np.uint32)
    crow256 = (p // 16 * 256).astype(np.uint32)
    joff = np.zeros((P, 3), np.uint32)
    cdiag = np.zeros((P, 4), np.uint32)
    cdiag[:, 0] = p * 16 + i16                      # st diag: rank p%16
    for g in range(3):
        valid = i16 < 6
        mrel = np.array(MREL)[np.minimum(i16, 5)]
        tmap = np.array(TMAP)[np.minimum(i16, 5)]
        cdiag[:, g + 1] = np.where(valid, p * 16 + 4 * g + mrel, p * 16 + 14)
        joff[:, g] = np.where(valid, (4 * g + tmap) * 1024, 15 * 1024)
    cu = np.zeros((P, 8), np.uint32)
    cu[:, 0] = cq1024
    cu[:, 1] = crow16
    cu[:, 2] = crow256
    cu[:, 3:6] = joff
    cu[:, 6] = cq1024 + 512
    return cu, cdiag


def _build16():
    nc = bacc.Bacc("TRN2", target_bir_lowering=False, debug=False,
                   num_devices=NCORES)
    z_d = nc.dram_tensor("z", [P, FREE], F32, kind="ExternalInput")
    cu_d = nc.dram_tensor("cu", [P, 8], U32, kind="ExternalInput")
    crb_d = nc.dram_tensor("crb", [P, 4], U32, kind="ExternalInput")
    softs_d = nc.dram_tensor("softs", [P * KF, 1], BF16, kind="ExternalOutput")
    st_d = nc.dram_tensor("st", [P * KF, 1], U8, kind="ExternalOutput")
    stgs_d = nc.dram_tensor("stgs", [P * 16, 1], U32, kind="Internal")
    stgg_d = nc.dram_tensor("stgg", [P * 16, 1], U32, kind="Internal")

    softs_2d = softs_d.ap().rearrange("(p f) o -> p (f o)", p=P)
    st_2d = st_d.ap().rearrange("(p f) o -> p (f o)", p=P)
    stgs_2d = stgs_d.ap().rearrange("(p c) o -> p (c o)", p=P)
    stgg_2d = stgg_d.ap().rearrange("(p c) o -> p (c o)", p=P)

    with tile.TileContext(nc) as tc, ExitStack() as ctx:
        sp = ctx.enter_context(tc.tile_pool(name="sp", bufs=1))

        # ---- tiles
        z = sp.tile([P, FREE], F32, tag="z")
        e0 = sp.tile([P, FREE], F32, tag="e0")
        etiles = {0: e0}
        for t in (4, 8, 12, 13, 14, 15):
            etiles[t] = sp.tile([P, FREE], F32, tag=f"e{t}", name=f"e{t}")
        softs_sb = sp.tile([P, KF], BF16, tag="softs_sb")
        stz = sp.tile([P, KF // 4], F32, tag="stz")      # u8 zeros via bitcast
        sel = sp.tile([P, 34], F32, tag="sel")
        selg = sel[:, 18:34].bitcast(U32)
        miu = sp.tile([P, 16], U32, tag="miu")
        cand = sp.tile([P, 16 * 34], F32, tag="cand")
        S0 = sp.tile([P, 1], F32, tag="S0")
        vbr = sp.tile([P, 32], F32, tag="vbr")
        ec = sp.tile([P, 256], F32, tag="ec")
        c2 = sp.tile([P, 256], F32, tag="c2")
        slotsg = sp.tile([P, 32], U32, tag="slotsg")
        cu = sp.tile([P, 8], U32, tag="cu")
        crbsrc = sp.tile([P, 4], U32, tag="crbsrc")
        crbd = sp.tile([P, 4], U32, tag="crbd")
        rbsraw = sp.tile([P, 4], U32, tag="rbsraw")
        rbsd = sp.tile([P, 4], U32, tag="rbsd")
        gdiag = sp.tile([P, 4], U32, tag="gdiag")
        gq = sp.tile([P, 4], U32, tag="gq")
        gm = sp.tile([P, 4], U32, tag="gm")
        pga = sp.tile([P, 4], U32, tag="pga")
        pgs = sp.tile([P, 4], U32, tag="pgs")
        obase = sp.tile([P, 4], U32, tag="obase")
        otmp = sp.tile([P, 4], U32, tag="otmp")
        offd = sp.tile([P, 4], U32, tag="offd")
        ones = sp.tile([P, 1], U8, tag="ones")
        zbf = sp.tile([P, 1], BF16, tag="zbf")
        padk = sp.tile([P, 48], F32, tag="padk")
        pf0 = sp.tile([P, 16], F32, tag="pf0")
        pf1 = sp.tile([P, 16], F32, tag="pf1")
        SSp = sp.tile([P, 16], F32, tag="SSp")

        # ---- inputs / consts
        nc.sync.dma_start(out=cu[:], in_=cu_d.ap())
        nc.sync.dma_start(out=crbsrc[:], in_=crb_d.ap())
        nc.scalar.dma_start(out=z[:, 0:H], in_=z_d.ap()[:, 0:H])
        nc.sync.dma_start(out=z[:, H:FREE], in_=z_d.ap()[:, H:FREE])

        # ---- st zero-fill (independent of input; DMA engines idle early)
        nc.vector.memset(stz[:], 0.0)
        nc.sync.dma_start(out=st_2d, in_=stz[:].bitcast(U8))
        # W-A-R token: waits for the zero DMA's completion sem
        nc.gpsimd.memset(stz[:, 0:1], 0.0)

        # ---- exp + per-partition selection
        nc.scalar.activation(e0[:, 0:H], z[:, 0:H], AF.Exp, scale=INV_TAU,
                             accum_out=sel[:, 16:17])
        nc.scalar.activation(e0[:, H:FREE], z[:, H:FREE], AF.Exp,
                             scale=INV_TAU, accum_out=sel[:, 17:18])
        nc.vector.max(sel[:, 0:8], e0[:, 0:H])
        nc.vector.max(sel[:, 8:16], e0[:, H:FREE])
        nc.vector.max_index(miu[:, 0:8], sel[:, 0:8], e0[:, 0:H])
        nc.vector.max_index(miu[:, 8:16], sel[:, 8:16], e0[:, H:FREE])
        # globalize candidate indices: + (p%16)*1024 (+512 for half B)
        nc.gpsimd.tensor_tensor(selg[:, 0:8], miu[:, 0:8],
                                cu[:, 0:1].to_broadcast([P, 8]), OP.add)
        nc.gpsimd.tensor_tensor(selg[:, 8:16], miu[:, 8:16],
                                cu[:, 6:7].to_broadcast([P, 8]), OP.add)

        # ---- per-row candidate gather via stream_shuffle
        for q in range(QP):
            nc.vector.stream_shuffle(cand[:, 34 * q:34 * q + 34], sel[:],
                                     [q] * 16 + [16 + q] * 16)
        gv = cand[:].rearrange("p (q c) -> p q c", c=34)

        nc.vector.tensor_reduce(S0[:], gv[:, :, 16:18],
                                axis=mybir.AxisListType.XY, op=OP.add)
        # row top-16 by value + slots
        nc.vector.max(vbr[:, 0:8], gv[:, :, 0:16])
        nc.vector.tensor_copy(ec[:].rearrange("p (q j) -> p q j", j=16),
                              gv[:, :, 0:16])
        nc.vector.max_index(slotsg[:, 0:8], vbr[:, 0:8], ec[:])
        nc.vector.match_replace(c2[:], vbr[:, 0:8], ec[:], 0.0)
        nc.vector.max(vbr[:, 8:16], c2[:])
        nc.vector.max_index(slotsg[:, 8:16], vbr[:, 8:16], c2[:])

        # ---- 1/S_j via prefix sums (gpsimd; serial chain of tiny ops)
        pf = [pf0, pf1]
        nc.gpsimd.tensor_copy(pf[0][:], vbr[:, 0:16])
        cur = 0
        for sh in (1, 2, 4, 8):
            nxt = 1 - cur
            nc.gpsimd.tensor_copy(pf[nxt][:, 0:sh], pf[cur][:, 0:sh])
            nc.gpsimd.tensor_tensor(pf[nxt][:, sh:16], pf[cur][:, sh:16],
                                    pf[cur][:, 0:16 - sh], OP.add)
            cur = nxt
        nc.gpsimd.tensor_scalar(SSp[:, 1:16], pf[cur][:, 0:15], -1.0, S0[:],
                                OP.mult, OP.add)
        nc.gpsimd.tensor_copy(SSp[:, 0:1], S0[:])
        nc.vector.reciprocal(vbr[:, 16:32], SSp[:])

        # ---- mr tree keys (pad with -1.0: never matches e > 0)
        nc.vector.memset(padk[:], -1.0)
        nc.vector.tensor_copy(padk[:, 0:4], vbr[:, 0:4])
        nc.vector.tensor_copy(padk[:, 8:12], vbr[:, 4:8])
        nc.vector.tensor_copy(padk[:, 16:20], vbr[:, 8:12])
        nc.vector.tensor_copy(padk[:, 24:25], vbr[:, 12:13])
        nc.vector.tensor_copy(padk[:, 32:33], vbr[:, 13:14])
        nc.vector.tensor_copy(padk[:, 40:41], vbr[:, 14:15])
        for i, (a, b) in enumerate([(0, 4), (4, 8), (8, 12), (12, 13),
                                    (13, 14), (14, 15)]):
            nc.vector.match_replace(etiles[b][:], padk[:, 8 * i:8 * i + 8],
                                    etiles[a][:], 0.0)

        # ---- scatter-offset machinery (slots -> rank-ordered gidx)
        nc.gpsimd.tensor_tensor(slotsg[:, 16:32], slotsg[:, 0:16],
                                cu[:, 2:3].to_broadcast([P, 16]), OP.add)
        nc.sync.dma_start(out=stgs_2d, in_=slotsg[:, 16:32])
        nc.sync.dma_start(out=stgg_2d, in_=selg[:, 0:16])
        # tokens: W-A-R into each staging DMA's read region
        nc.gpsimd.memset(slotsg[:, 16:17], 0)
        nc.gpsimd.memset(sel[:, 18:19], 0.0)
        # crbd = crb + 0*tok_s  (value-preserving, carries stgs completion)
        nc.gpsimd.tensor_tensor(crbd[:], crbsrc[:],
                                slotsg[:, 16:17].to_broadcast([P, 4]), OP.add)
        for x in range(4):
            nc.gpsimd.indirect_dma_start(
                out=rbsraw[:, x:x + 1], out_offset=None, in_=stgs_d.ap(),
                in_offset=bass.IndirectOffsetOnAxis(ap=crbd[:, x:x + 1],
                                                    axis=0))
        # rbsd = rbsraw + 0*tok_g (carries stgg completion)
        nc.gpsimd.tensor_tensor(rbsd[:], rbsraw[:],
                                sel[:, 18:19].bitcast(U32).to_broadcast([P, 4]),
                                OP.add)
        for x in range(4):
            nc.gpsimd.indirect_dma_start(
                out=gdiag[:, x:x + 1], out_offset=None, in_=stgg_d.ap(),
                in_offset=bass.IndirectOffsetOnAxis(ap=rbsd[:, x:x + 1],
                                                    axis=0))
        # offsets: ((g>>10) + 16*row) << 14  +  j*1024  +  (g & 1023)
        nc.vector.tensor_scalar(gq[:], gdiag[:], 10, None,
                                OP.logical_shift_right)
        nc.vector.tensor_scalar(gm[:], gdiag[:], 1023, None, OP.bitwise_and)
        nc.gpsimd.tensor_tensor(pga[:], gq[:],
                                cu[:, 1:2].to_broadcast([P, 4]), OP.add)
        nc.gpsimd.tensor_scalar(pgs[:], pga[:], KF, None, OP.mult)
        nc.gpsimd.tensor_tensor(obase[:], pgs[:], gm[:], OP.add)
        # col 0: st (j = p%16); cols 1..3: soft groups (j const map)
        nc.gpsimd.tensor_tensor(otmp[:, 0:1], obase[:, 0:1], cu[:, 0:1],
                                OP.add)
        for g in range(3):
            nc.gpsimd.tensor_tensor(otmp[:, g + 1:g + 2],
                                    obase[:, g + 1:g + 2],
                                    cu[:, 3 + g:4 + g], OP.add)

        nc.vector.memset(ones[:], 1)
        nc.vector.memset(zbf[:], 0.0)

        # st scatter: + 0*tok_z (carries st zero-fill completion)
        nc.gpsimd.tensor_tensor(offd[:, 0:1], otmp[:, 0:1],
                                stz[:, 0:1].bitcast(U32), OP.add)
        nc.gpsimd.indirect_dma_start(
            out=st_d.ap(),
            out_offset=bass.IndirectOffsetOnAxis(ap=offd[:, 0:1], axis=0),
            in_=ones[:], in_offset=None)

        # ---- soft planes + group DMAs + fix-up scatters
        act_planes = {0, 1, 2, 3, 8, 9, 10, 11, 13, 15}
        src_of = {j: (j // 4) * 4 for j in range(12)}
        src_of.update({12: 12, 13: 13, 14: 14, 15: 15})
        for g in range(4):
            for j in range(4 * g, 4 * g + 4):
                dst = softs_sb[:, j * FREE:(j + 1) * FREE]
                src = etiles[src_of[j]][:]
                scl = vbr[:, 16 + j:17 + j]
                if j in act_planes:
                    nc.scalar.activation(dst, src, AF.Copy, scale=scl)
                else:
                    nc.vector.tensor_scalar(dst, src, scl, None, OP.mult)
            nc.sync.dma_start(out=softs_2d[:, 4 * g * FREE:(4 * g + 4) * FREE],
                              in_=softs_sb[:, 4 * g * FREE:(4 * g + 4) * FREE])
            if g < 3:
                # W-A-R token into the group's (now dead) SBUF region
                nc.gpsimd.memset(softs_sb[:, 4 * g * FREE:4 * g * FREE + 2],
                                 0.0)
                tok = softs_sb[:, 4 * g * FREE:4 * g * FREE + 2].bitcast(U32)
                nc.gpsimd.tensor_tensor(offd[:, g + 1:g + 2],
                                        otmp[:, g + 1:g + 2], tok, OP.add)
                nc.gpsimd.indirect_dma_start(
                    out=softs_d.ap(),
                    out_offset=bass.IndirectOffsetOnAxis(
                        ap=offd[:, g + 1:g + 2], axis=0),
                    in_=zbf[:], in_offset=None)
    nc.compile()
    return nc


def kernel(logits, gumbel, k, trace=False):
    K = int(k)
    logits = np.ascontiguousarray(logits, dtype=np.float32)
    gumbel = np.ascontiguousarray(gumbel, dtype=np.float32)
    if K == 0:
        empty = np.zeros((0, B, N), dtype=np.float32)
        return empty, empty.copy()
    assert K == 16, f"v2 kernel supports k=16 only, got {K}"
    assert logits.shape == (B, N) and gumbel.shape == (B, N)

    if K not in _module_cache:
        _module_cache[K] = _build16()
    nc = _module_cache[K]

    cu, crb = _host_consts()
    z_full = logits + gumbel
    in_maps = []
    for c in range(NCORES):
        sl = slice(c * R, (c + 1) * R)
        in_maps.append({"z": z_full[sl].reshape(P, FREE),
                        "cu": cu, "crb": crb})

    res = run_bass_kernel_spmd(nc, in_maps, core_ids=list(range(NCORES)),
                               trace=trace)

    st = np.empty((K, B, N), dtype=np.float32)
    softs = np.empty((K, B, N), dtype=np.float32)
    for c in range(NCORES):
        sl = slice(c * R, (c + 1) * R)
        s = res.results[c]["softs"].reshape(R, QP, K, FREE)
        softs[:, sl, :] = np.transpose(s.astype(np.float32), (2, 0, 1, 3)) \
            .reshape(K, R, N)
        t = res.results[c]["st"].reshape(R, QP, K, FREE)
        st[:, sl, :] = np.transpose(t, (2, 0, 1, 3)).reshape(K, R, N) \
            .astype(np.float32)

    if trace:
        kernel.last_exec_time_ns = res.exec_time_ns
        kernel.last_results = res
    return st, softs


# revision 8
# speedup vs baseline: 1.4627x; 1.4627x over previous
"""Gumbel top-k (sequential masking) Trainium2 kernel, v3.

B=64 rows, N=16384, K=16 sequential top-1+mask steps; outputs st
(one-hot) and softs, each [K, B, N] f32 (softs emitted bf16, st u8).

Data-parallel over batch: 8 rows/core x 8 cores; row = 16 partitions
x 1024. DRAM outputs are partition-major [P, K*1024] (fat contiguous
per-partition DMA runs); host transposes back.

Key structure:
  - st = zero-planes (4 DMAs from one zeroed SBUF block) + 128
    scattered one-bytes via a single [P,1] indirect DMA.
  - softs plane j = e_src * (1/S_j) with bf16 output on ACT/DVE.
    e0 -> e4 -> e8 via match_replace (4 keys), planes 8..15 get exact
    tiles via a depth-1 fan-out from e8 (keys vbr[8:j]).  Planes
    1..3/5..7 are written group-masked and their <=3 stale positions
    per row are zeroed in DRAM by ONE [P,1] indirect scatter
    (q-packed items; idle q-slots write an idempotent 0 to plane15's
    rank-14 position).
  - Scatter offsets come from a premultiplied staging table: each
    partition stages p*16384 + (half*512 + max_index) per candidate
    plus a row-level slot->flat pointer table; two chained [P,1]
    gathers (const diagonal maps, host-precomputed) resolve
    rank-ordered absolute element offsets with NO late arithmetic
    except one add of a constant plane offset.
  - DRAM W-A-W ordering (zero/plane DMA before scatter) via Tile
    W-A-R: a tiny memset into the DMA's read region waits on the DMA
    completion sem; a value-preserving add of that (zero) token into
    the consumer's offset tile carries the dependency.
  - 1/S_j split: 1/S0 immediately after the row-sum; ranks 1-7 after
    the first row max8; ranks 8-15 after the second — so early planes
    start as soon as possible.
"""

import numpy as np
from contextlib import ExitStack

import concourse.bacc as bacc
import concourse.bass as bass
import concourse.mybir as mybir
import concourse.tile as tile
from concourse.bass_utils import run_bass_kernel_spmd

F32 = mybir.dt.float32
BF16 = mybir.dt.bfloat16
U8 = mybir.dt.uint8
U32 = mybir.dt.uint32
AF = mybir.ActivationFunctionType
OP = mybir.AluOpType

B, N, NCORES = 64, 16384, 8
R = B // NCORES          # rows per core = 8
QP = 16                  # partitions per row
FREE = N // QP           # 1024
P = 128                  # SBUF partitions
H = FREE // 2            # 512
INV_TAU = 1.5            # 1/(2/3), exact in fp32
K16 = 16
KF = K16 * FREE          # elements per partition of each output

# scatter-fixed groups 0,1: item q -> (plane offset, rank offset)
TMAP = [1, 2, 2, 3, 3, 3]
MREL = [0, 0, 1, 0, 1, 2]

ACT_PLANES = {0, 1, 2, 3, 5, 6, 7, 9, 10, 11, 13, 15}

_module_cache = {}


def _host_consts():
    p = np.arange(P)
    q = p % 16
    cc = np.zeros((P, 8), np.uint32)
    cc[:, 0] = p * 16384                     # premult base, half A
    cc[:, 1] = p * 16384 + 512               # premult base, half B
    cc[:, 2] = (p // 16) * 512               # slot-flat row base
    cc[:, 3] = p * 32 + 16 + q               # diag map: st (rank q)
    # diag map: soft zeros, q-packed items
    m = np.where(q < 6, np.array(MREL * 3)[q],
                 np.where(q < 12, 4 + np.array(MREL * 3)[q - 6], 14))
    cc[:, 4] = p * 32 + 16 + m
    cc[:, 5] = q * 1024                      # st plane offset (j = q)
    j = np.where(q < 6, np.array(TMAP * 3)[q],
                 np.where(q < 12, 4 + np.array(TMAP * 3)[q - 6], 15))
    cc[:, 6] = j * 1024                      # soft plane offset
    return cc


def _build16():
    nc = bacc.Bacc("TRN2", target_bir_lowering=False, debug=False,
                   num_devices=NCORES)
    z_d = nc.dram_tensor("z", [P, FREE], F32, kind="ExternalInput")
    cc_d = nc.dram_tensor("cc", [P, 8], U32, kind="ExternalInput")
    softs_d = nc.dram_tensor("softs", [P * KF, 1], BF16, kind="ExternalOutput")
    st_d = nc.dram_tensor("st", [P * KF, 1], U8, kind="ExternalOutput")
    stg_d = nc.dram_tensor("stg", [P * 32, 1], U32, kind="Internal")

    softs_2d = softs_d.ap().rearrange("(p f) o -> p (f o)", p=P)
    st_2d = st_d.ap().rearrange("(p f) o -> p (f o)", p=P)
    stg_2d = stg_d.ap().rearrange("(p c) o -> p (c o)", p=P)

    with tile.TileContext(nc) as tc, ExitStack() as ctx:
        sp = ctx.enter_context(tc.tile_pool(name="sp", bufs=1))

        # ---- tiles
        z = sp.tile([P, FREE], F32, tag="z")
        e0 = sp.tile([P, FREE], F32, tag="e0")
        etiles = {0: e0}
        for t in (4, 8, 9, 10, 11, 12, 13, 14, 15):
            etiles[t] = sp.tile([P, FREE], F32, tag=f"e{t}", name=f"e{t}")
        softs_sb = sp.tile([P, KF], BF16, tag="softs_sb")
        stz = sp.tile([P, FREE], F32, tag="stz")   # 4KB of u8 zeros via view
        sel = sp.tile([P, 18], F32, tag="sel")     # 16 max vals + 2 accums
        miu = sp.tile([P, 16], U32, tag="miu")
        scomb = sp.tile([P, 32], U32, tag="scomb")  # staged: premult+slotflat
        cand = sp.tile([P, 16 * 18], F32, tag="cand")
        S0 = sp.tile([P, 1], F32, tag="S0")
        vbr = sp.tile([P, 32], F32, tag="vbr")
        ec = sp.tile([P, 256], F32, tag="ec")
        c2 = sp.tile([P, 256], F32, tag="c2")
        slots = sp.tile([P, 16], U32, tag="slots")
        sadj = sp.tile([P, 48], U32, tag="sadj")
        cc = sp.tile([P, 8], U32, tag="cc")
        crbd = sp.tile([P, 2], U32, tag="crbd")
        rbs = sp.tile([P, 2], U32, tag="rbs")
        ob = sp.tile([P, 2], U32, tag="ob")
        otmp = sp.tile([P, 2], U32, tag="otmp")
        offd = sp.tile([P, 2], U32, tag="offd")
        ones = sp.tile([P, 1], U8, tag="ones")
        zbf = sp.tile([P, 1], BF16, tag="zbf")
        padk = sp.tile([P, 72], F32, tag="padk")
        pfa0 = sp.tile([P, 8], F32, tag="pfa0")
        pfa1 = sp.tile([P, 8], F32, tag="pfa1")
        pfb0 = sp.tile([P, 8], F32, tag="pfb0")
        pfb1 = sp.tile([P, 8], F32, tag="pfb1")
        SSp = sp.tile([P, 16], F32, tag="SSp")
        SA = sp.tile([P, 1], F32, tag="SA")

        # ---- inputs / consts / st zero-fill
        nc.sync.dma_start(out=cc[:], in_=cc_d.ap())
        nc.scalar.dma_start(out=z[:, 0:H], in_=z_d.ap()[:, 0:H])
        nc.sync.dma_start(out=z[:, H:FREE], in_=z_d.ap()[:, H:FREE])
        nc.gpsimd.memset(stz[:], 0.0)
        for g in range(4):
            nc.sync.dma_start(out=st_2d[:, g * 4096:(g + 1) * 4096],
                              in_=stz[:].bitcast(U8))
        # W-A-R token: waits for all four zero DMAs' completion
        nc.gpsimd.memset(stz[:, 0:1], 0.0)

        # ---- exp + per-partition selection
        nc.scalar.activation(e0[:, 0:H], z[:, 0:H], AF.Exp, scale=INV_TAU,
                             accum_out=sel[:, 16:17])
        nc.scalar.activation(e0[:, H:FREE], z[:, H:FREE], AF.Exp,
                             scale=INV_TAU, accum_out=sel[:, 17:18])
        nc.vector.max(sel[:, 0:8], e0[:, 0:H])
        nc.vector.max(sel[:, 8:16], e0[:, H:FREE])
        nc.vector.max_index(miu[:, 0:8], sel[:, 0:8], e0[:, 0:H])
        nc.vector.max_index(miu[:, 8:16], sel[:, 8:16], e0[:, H:FREE])
        # premultiplied candidate offsets: p*16384 + (half*512 + idx)
        nc.gpsimd.tensor_tensor(scomb[:, 0:8], miu[:, 0:8],
                                cc[:, 0:1].to_broadcast([P, 8]), OP.add)
        nc.gpsimd.tensor_tensor(scomb[:, 8:16], miu[:, 8:16],
                                cc[:, 1:2].to_broadcast([P, 8]), OP.add)

        # ---- per-row candidate values via stream_shuffle
        for q in range(QP):
            nc.vector.stream_shuffle(cand[:, 18 * q:18 * q + 18], sel[:],
                                     [q] * 16 + [16 + q] * 16)
        gv = cand[:].rearrange("p (q c) -> p q c", c=18)

        nc.vector.tensor_reduce(S0[:], gv[:, :, 16:18],
                                axis=mybir.AxisListType.XY, op=OP.add)
        nc.vector.reciprocal(vbr[:, 16:17], S0[:])      # r_0 early
        # row top-16 by value + slots
        nc.vector.max(vbr[:, 0:8], gv[:, :, 0:16])
        nc.vector.tensor_copy(ec[:].rearrange("p (q j) -> p q j", j=16),
                              gv[:, :, 0:16])
        nc.vector.max_index(slots[:, 0:8], vbr[:, 0:8], ec[:])
        nc.vector.match_replace(c2[:], vbr[:, 0:8], ec[:], 0.0)
        nc.vector.max(vbr[:, 8:16], c2[:])
        nc.vector.max_index(slots[:, 8:16], vbr[:, 8:16], c2[:])
        # slot s=(16q+c) -> flat pointer 512*row + 32q + c = 2s - (s&15) + 512row
        nc.vector.tensor_scalar(sadj[:, 0:16], slots[:], 1, None,
                                OP.logical_shift_left)
        nc.vector.tensor_scalar(sadj[:, 16:32], slots[:], 15, None,
                                OP.bitwise_and)
        nc.vector.tensor_tensor(sadj[:, 32:48], sadj[:, 0:16],
                                sadj[:, 16:32], OP.subtract)
        nc.gpsimd.tensor_tensor(scomb[:, 16:32], sadj[:, 32:48],
                                cc[:, 2:3].to_broadcast([P, 16]), OP.add)

        # ---- staging + token + diag maps
        nc.sync.dma_start(out=stg_2d, in_=scomb[:])
        nc.gpsimd.memset(scomb[:, 16:17], 0)
        nc.gpsimd.tensor_tensor(crbd[:], cc[:, 3:5],
                                scomb[:, 16:17].to_broadcast([P, 2]), OP.add)

        # ---- 1/S_j: ranks 1-7 after g1, ranks 8-15 after g2
        pfa = [pfa0, pfa1]
        nc.gpsimd.tensor_copy(pfa[0][:], vbr[:, 0:8])
        cur = 0
        for sh in (1, 2, 4):
            nxt = 1 - cur
            nc.gpsimd.tensor_copy(pfa[nxt][:, 0:sh], pfa[cur][:, 0:sh])
            nc.gpsimd.tensor_tensor(pfa[nxt][:, sh:8], pfa[cur][:, sh:8],
                                    pfa[cur][:, 0:8 - sh], OP.add)
            cur = nxt
        nc.gpsimd.tensor_scalar(SSp[:, 1:8], pfa[cur][:, 0:7], -1.0, S0[:],
                                OP.mult, OP.add)
        nc.vector.reciprocal(vbr[:, 17:24], SSp[:, 1:8])
        nc.gpsimd.tensor_scalar(SA[:], pfa[cur][:, 7:8], -1.0, S0[:],
                                OP.mult, OP.add)          # S_8 = S0 - sum v0..7
        pfb = [pfb0, pfb1]
        nc.gpsimd.tensor_copy(pfb[0][:], vbr[:, 8:16])
        cur = 0
        for sh in (1, 2, 4):
            nxt = 1 - cur
            nc.gpsimd.tensor_copy(pfb[nxt][:, 0:sh], pfb[cur][:, 0:sh])
            nc.gpsimd.tensor_tensor(pfb[nxt][:, sh:8], pfb[cur][:, sh:8],
                                    pfb[cur][:, 0:8 - sh], OP.add)
            cur = nxt
        nc.gpsimd.tensor_copy(SSp[:, 8:9], SA[:])
        nc.gpsimd.tensor_scalar(SSp[:, 9:16], pfb[cur][:, 0:7], -1.0, SA[:],
                                OP.mult, OP.add)
        nc.vector.reciprocal(vbr[:, 24:32], SSp[:, 8:16])

        # ---- mr tree: e4, e8, then depth-1 fan-out e9..e15 from e8
        nc.vector.memset(padk[:], -1.0)
        nc.vector.tensor_copy(padk[:, 0:4], vbr[:, 0:4])
        nc.vector.tensor_copy(padk[:, 8:12], vbr[:, 4:8])
        for i, j in enumerate(range(9, 16)):
            nc.vector.tensor_copy(padk[:, 16 + 8 * i:16 + 8 * i + (j - 8)],
                                  vbr[:, 8:j])
        nc.vector.match_replace(etiles[4][:], padk[:, 0:8], e0[:], 0.0)
        nc.vector.match_replace(etiles[8][:], padk[:, 8:16], etiles[4][:],
                                0.0)
        for i, j in enumerate(range(9, 16)):
            nc.vector.match_replace(etiles[j][:], padk[:, 16 + 8 * i:24 + 8 * i],
                                    etiles[8][:], 0.0)

        # ---- soft planes + group DMAs (all emitted before indirects)
        def src_of(j):
            return etiles[0 if j < 4 else (4 if j < 8 else j)]

        for g in range(4):
            for j in range(4 * g, 4 * g + 4):
                dst = softs_sb[:, j * FREE:(j + 1) * FREE]
                scl = vbr[:, 16 + j:17 + j]
                if j in ACT_PLANES:
                    nc.scalar.activation(dst, src_of(j)[:], AF.Copy, scale=scl)
                else:
                    nc.vector.tensor_scalar(dst, src_of(j)[:], scl, None,
                                            OP.mult)
            nc.sync.dma_start(out=softs_2d[:, 4 * g * FREE:(4 * g + 4) * FREE],
                              in_=softs_sb[:, 4 * g * FREE:(4 * g + 4) * FREE])

        # W-A-R tokens for groups 0,1 (their DRAM gets scatter-fixed)
        nc.gpsimd.memset(softs_sb[:, 0:2], 0.0)
        nc.gpsimd.memset(softs_sb[:, 4 * FREE:4 * FREE + 2], 0.0)

        # ---- indirect block: 2 id-gathers, 2 gathers, 2 scatters
        nc.vector.memset(ones[:], 1)
        nc.vector.memset(zbf[:], 0.0)
        for x in range(2):
            nc.gpsimd.indirect_dma_start(
                out=rbs[:, x:x + 1], out_offset=None, in_=stg_d.ap(),
                in_offset=bass.IndirectOffsetOnAxis(ap=crbd[:, x:x + 1],
                                                    axis=0))
        for x in range(2):
            nc.gpsimd.indirect_dma_start(
                out=ob[:, x:x + 1], out_offset=None, in_=stg_d.ap(),
                in_offset=bass.IndirectOffsetOnAxis(ap=rbs[:, x:x + 1],
                                                    axis=0))
        # st: + j*1024 (j=q) + 0*tok_z
        nc.gpsimd.tensor_tensor(otmp[:, 0:1], ob[:, 0:1], cc[:, 5:6], OP.add)
        nc.gpsimd.tensor_tensor(offd[:, 0:1], otmp[:, 0:1],
                                stz[:, 0:1].bitcast(U32), OP.add)
        nc.gpsimd.indirect_dma_start(
            out=st_d.ap(),
            out_offset=bass.IndirectOffsetOnAxis(ap=offd[:, 0:1], axis=0),
            in_=ones[:], in_offset=None)
        # soft zeros: + joff + 0*tok_g0 + 0*tok_g1
        nc.gpsimd.tensor_tensor(otmp[:, 1:2], ob[:, 1:2], cc[:, 6:7], OP.add)
        nc.gpsimd.tensor_tensor(otmp[:, 1:2], otmp[:, 1:2],
                                softs_sb[:, 0:2].bitcast(U32), OP.add)
        nc.gpsimd.tensor_tensor(offd[:, 1:2], otmp[:, 1:2],
                                softs_sb[:, 4 * FREE:4 * FREE + 2]
                                .bitcast(U32), OP.add)
        nc.gpsimd.indirect_dma_start(
            out=softs_d.ap(),
            out_offset=bass.IndirectOffsetOnAxis(ap=offd[:, 1:2], axis=0),
            in_=zbf[:], in_offset=None)
    nc.compile()
    return nc


def kernel(logits, gumbel, k, trace=False):
    K = int(k)
    logits = np.ascontiguousarray(logits, dtype=np.float32)
    gumbel = np.ascontiguousarray(gumbel, dtype=np.float32)
    if K == 0:
        empty = np.zeros((0, B, N), dtype=np.float32)
        return empty, empty.copy()
    assert K == 16, f"kernel supports k=16 only, got {K}"
    assert logits.shape == (B, N) and gumbel.shape == (B, N)

    if K not in _module_cache:
        _module_cache[K] = _build16()
    nc = _module_cache[K]

    cc = _host_consts()
    z_full = logits + gumbel
    in_maps = []
    for c in range(NCORES):
        sl = slice(c * R, (c + 1) * R)
        in_maps.append({"z": z_full[sl].reshape(P, FREE), "cc": cc})

    res = run_bass_kernel_spmd(nc, in_maps, core_ids=list(range(NCORES)),
                               trace=trace)

    st = np.empty((K, B, N), dtype=np.float32)
    softs = np.empty((K, B, N), dtype=np.float32)
    for c in range(NCORES):
        sl = slice(c * R, (c + 1) * R)
        s = res.results[c]["softs"].reshape(R, QP, K, FREE)
        softs[:, sl, :] = np.transpose(s.astype(np.float32), (2, 0, 1, 3)) \
            .reshape(K, R, N)
        t = res.results[c]["st"].reshape(R, QP, K, FREE)
        st[:, sl, :] = np.transpose(t, (2, 0, 1, 3)).reshape(K, R, N) \
            .astype(np.float32)

    if trace:
        kernel.last_exec_time_ns = res.exec_time_ns
        kernel.last_results = res
    return st, softs
